# revision 15
# baseline (speedup 1.0000x reference)
"""Causal self-attention (B=2, S=2048, D=1024, H=16) on 8 Trainium2 cores.

Sharding: batch x head-group. Core c handles batch c//4 and heads
[4*(c%4), 4*(c%4)+4). Each core computes q/k/v projections for its head
slice, causal flash-attention (transposed layout, no max-subtraction --
scores are bounded ~9), and a row-parallel partial output projection.
The host transposes/sums the 8 partial outputs and adds the folded bias
(b_proj + w_proj @ b_v -- the v bias commutes through softmax).

All attention + projection matmuls run in bf16 (1 cyc/row on the PE).
Partial outputs are written as fp16 (halves the write traffic; partials
are |y|<~10 so fp16 rounding is ~4e-4 absolute per partial).
"""

import sys

import numpy as np

try:
    import concourse.bass as bass  # noqa: F401
except ImportError:  # fallback for environments without the site hook
    sys.path.insert(0, "/opt/trn_rl_repo")

import concourse.bacc as bacc
import concourse.bass as bass
import concourse.mybir as mybir
from concourse import tile
from concourse.bass_utils import run_bass_kernel_spmd

B, S, D, H = 2, 2048, 1024, 16
HD = D // H  # 64
SCALE = 1.0 / np.sqrt(HD)  # 0.125
HPC = 4          # heads per core
NCORES = 8
P = 128          # partitions
QC = 512         # query chunk (matmul free dim)
NQ = S // QC     # 4 query chunks
NK = S // P      # 16 key tiles
ND = D // P      # 8 d tiles
F32 = mybir.dt.float32
F16 = mybir.dt.float16
BF16 = mybir.dt.bfloat16
ATT_DT = BF16
VPAD = 336                      # v tile cols: 4*65 rounded up so every
                                # head slice can read a full 128-col lhsT
N_WARM = 4                      # dummy matmuls to ramp the PE p-state

_PROGRAM = None


def _build_program():
    """Build the SPMD Bass program (same NEFF for all 8 cores)."""
    nc = bacc.Bacc(None, target_bir_lowering=False)

    xt = nc.declare_dram_parameter("xt", [D, S], ATT_DT, isOutput=False)
    wqk = nc.declare_dram_parameter("wqk", [D, 4 * P], ATT_DT, isOutput=False)
    wv = nc.declare_dram_parameter("wv", [D, HPC * HD], ATT_DT, isOutput=False)
    bqk = nc.declare_dram_parameter("bqk", [P, 4], F32, isOutput=False)
    masks = nc.declare_dram_parameter("masks", [P, 2 * P], ATT_DT, isOutput=False)
    wp = nc.declare_dram_parameter("wp", [HPC * HD, D], ATT_DT, isOutput=False)
    yt = nc.declare_dram_parameter("yt", [D, S], F16, isOutput=True)

    VW = HPC * HD  # 256 cols of v (no bias/ones columns in DRAM)

    with tile.TileContext(nc) as tc:
        with (
            tc.tile_pool(name="const", bufs=1) as const,
            tc.tile_pool(name="big", bufs=1) as bigp,
            tc.tile_pool(name="ps_mm", bufs=2, space="PSUM") as ps_mm,
            tc.tile_pool(name="ps_pv", bufs=4, space="PSUM") as ps_pv,
        ):
            xtp_cm = tc.tile_pool(name="xtp", bufs=1)
            xtp = xtp_cm.__enter__()

            # explicit 2-queue DMA schedule (sync + gpsimd; scalar must
            # stay free for activations). Each dma_start is striped over
            # all 16 physical DMA engines, so fewer+bigger transfers win.
            # ---- PE warm-up: dummy matmuls on a memset tile ramp the
            # p-state while the first DMAs are in flight ----
            warm_sb = const.tile([P, QC], ATT_DT, tag="warm")
            nc.vector.memset(warm_sb[:], 0.0)
            warm_ps = ps_mm.tile([P, QC], F32, tag="mm", name="warm")
            for _ in range(N_WARM):
                nc.tensor.matmul(warm_ps[:], warm_sb[:, 0:P], warm_sb[:],
                                 start=True, stop=True)

            wqk_all = const.tile([P, ND * 4 * P], ATT_DT, tag="wqk")
            wqk_sb = [wqk_all[:, dt * 4 * P:(dt + 1) * 4 * P] for dt in range(ND)]
            xt_sb = [
                xtp.tile([P, S], ATT_DT, tag=f"xt{dt}", name=f"xts{dt}")
                for dt in range(ND)
            ]
            bqk_sb = const.tile([P, 4], F32, tag="bqk")
            nc.gpsimd.dma_start(bqk_sb[:], bqk[:])
            wqk_src = wqk[:].rearrange("(d p) c -> p d c", d=ND)
            wqk_dst = wqk_all[:].rearrange("p (d c) -> p d c", d=ND)
            # wqk by et-quarter (all d-tiles of one 128-col weight block per
            # transfer) so the first q/k chains start after 1/4 of wqk
            for et in range(4):
                eng = nc.sync if et % 2 == 0 else nc.gpsimd
                eng.dma_start(wqk_dst[:, :, et * P:(et + 1) * P],
                              wqk_src[:, :, et * P:(et + 1) * P])
            for dt in range(ND):
                eng = nc.sync if dt % 2 == 0 else nc.gpsimd
                eng.dma_start(xt_sb[dt][:, 0:QC], xt[dt * P:(dt + 1) * P, 0:QC])
            # rest of x: sc1 first (consumed next), then sc2+sc3
            for dt in range(ND):
                eng = nc.sync if dt % 2 == 0 else nc.gpsimd
                eng.dma_start(xt_sb[dt][:, QC:2 * QC],
                              xt[dt * P:(dt + 1) * P, QC:2 * QC])
            for dt in range(ND):
                eng = nc.sync if dt % 2 == 0 else nc.gpsimd
                eng.dma_start(xt_sb[dt][:, 2 * QC:S],
                              xt[dt * P:(dt + 1) * P, 2 * QC:S])
            wv_all = const.tile([P, ND * VW], ATT_DT, tag="wv")
            wv_sb = [wv_all[:, dt * VW:(dt + 1) * VW] for dt in range(ND)]
            nc.gpsimd.dma_start(
                wv_all[:].rearrange("p (d c) -> p d c", d=ND),
                wv[:].rearrange("(d p) c -> p d c", d=ND),
            )
            masks_sb = const.tile([P, 2 * P], ATT_DT, tag="masks")
            nc.sync.dma_start(masks_sb[:], masks[:])
            wp_all = const.tile([P, 2 * D], ATT_DT, tag="wp")
            wp_sb = [wp_all[:, i * D:(i + 1) * D] for i in range(2)]
            nc.gpsimd.dma_start(
                wp_all[:].rearrange("p (i c) -> p i c", i=2),
                wp[:].rearrange("(i p) c -> p i c", i=2),
            )

            # ---- persistent intermediates ----
            qt_sb = [bigp.tile([P, S], ATT_DT, tag=f"qt{i}", name=f"qt{i}") for i in range(2)]
            kt_sb = [bigp.tile([P, S], ATT_DT, tag=f"kt{i}", name=f"kt{i}") for i in range(2)]
            v_sb = [bigp.tile([P, VPAD], ATT_DT, tag=f"v{i}", name=f"v{i}") for i in range(NK)]
            ot_sb = [bigp.tile([P, S], ATT_DT, tag=f"ot{i}", name=f"ot{i}") for i in range(2)]

            # ones everywhere except the 4x64 blocks the copies fill:
            # column 65h+64 of each head block stays 1 => the pv matmul's
            # 65-stride window trick yields the softmax denominator row.
            for st in range(NK):
                nc.gpsimd.memset(v_sb[st][:], 1.0)

            # ================= phase 1: q/k projections =================
            for sc in range(NQ):
                for et in range(4):  # 0,1: q heads (0,1),(2,3); 2,3: k heads
                    ps = ps_mm.tile([P, QC], F32, tag="mm", name=f"qk{sc}{et}")
                    for dt in range(ND):
                        nc.tensor.matmul(
                            ps[:],
                            wqk_sb[dt][:, et * P:(et + 1) * P],
                            xt_sb[dt][:, sc * QC:(sc + 1) * QC],
                            start=(dt == 0),
                            stop=(dt == ND - 1),
                        )
                    dest = (qt_sb if et < 2 else kt_sb)[et % 2]
                    dst_ap = dest[:, sc * QC:(sc + 1) * QC]
                    nc.vector.tensor_scalar_add(dst_ap, ps[:], bqk_sb[:, et:et + 1])

            # ================= phase 1b: v projection =================
            def emit_v(st):
                ps = ps_mm.tile([P, VW], F32, tag="mm", name=f"vp{st}")
                for dt in range(ND):
                    nc.tensor.matmul(
                        ps[:],
                        xt_sb[dt][:, st * P:(st + 1) * P],
                        wv_sb[dt][:],
                        start=(dt == 0),
                        stop=(dt == ND - 1),
                    )
                # scatter the 4 64-wide head blocks into the 65-stride
                # layout, skipping the ones columns
                dst = v_sb[st][:, 0:4 * 65].rearrange("p (h d) -> p h d", h=4)[:, :, 0:HD]
                src = ps[:].rearrange("p (h d) -> p h d", h=4)
                if st % 2 == 0:
                    nc.scalar.copy(dst, src)
                else:
                    nc.vector.tensor_copy(dst, src)

            for st in range(4):
                emit_v(st)

            work_cm = tc.tile_pool(name="work", bufs=6)
            work = work_cm.__enter__()
            small_cm = tc.tile_pool(name="small", bufs=3)
            small = small_cm.__enter__()

            # ================= phase 2: attention =================
            def emit_pair(qt, pair, fillers=None, split_exp=False):
                q0 = qt * QC
                nk = (qt + 1) * (QC // P)  # causal: k tiles 0..nk-1
                ht = pair
                pvs = [
                    ps_pv.tile([P, QC], F32, tag="pv", name=f"pv{qt}{pair}{hh}")
                    for hh in range(2)
                ]
                for ki, kb in enumerate(range(nk)):
                    j = kb - qt * (QC // P)
                    # diagonal strip: columns < 128*j are fully masked
                    off = 0 if j < 0 else P * j
                    w = QC - off
                    st2 = ps_mm.tile(
                        [P, 2 * QC], F32, tag="mm", name=f"st{qt}{pair}{kb}"
                    )
                    for hh in range(2):
                        nc.tensor.matmul(
                            st2[:, hh * QC + off:(hh + 1) * QC],
                            kt_sb[ht][slice(64 * hh, 64 * hh + 64),
                                      kb * P:(kb + 1) * P],
                            qt_sb[ht][slice(64 * hh, 64 * hh + 64),
                                      q0 + off:q0 + QC],
                            start=True, stop=True,
                            tile_position=(64 * hh, 0),
                        )
                    ex = work.tile(
                        [P, 2 * QC], ATT_DT, tag="ex", name=f"ex{qt}{pair}{kb}"
                    )
                    st3 = st2[:].rearrange("p (h q) -> p h q", h=2)[:, :, off:]
                    ex3 = ex[:].rearrange("p (h q) -> p h q", h=2)[:, :, off:]
                    if split_exp and j >= 0:
                        # final pair: halve the serial exp latency per tile
                        for hh in range(2):
                            nc.scalar.activation(
                                ex3[:, hh:hh + 1, :], st3[:, hh:hh + 1, :],
                                mybir.ActivationFunctionType.Exp,
                                scale=float(SCALE),
                            )
                    else:
                        nc.scalar.activation(
                            ex3, st3,
                            mybir.ActivationFunctionType.Exp,
                            scale=float(SCALE),
                        )
                    if j >= 0:
                        # only the leading 128 columns of the window straddle
                        # the diagonal; the rest is fully unmasked
                        exm = ex[:].rearrange("p (h q) -> p h q", h=2)[:, :, off:off + P]
                        m3 = masks_sb[:].rearrange("p (h q) -> p h q", h=2)
                        nc.vector.tensor_mul(exm, exm, m3)
                    for hh in range(2):
                        h = 2 * pair + hh
                        nc.tensor.matmul(
                            pvs[hh][:, off:],
                            v_sb[kb][:, h * (HD + 1):h * (HD + 1) + P],
                            ex[:, hh * QC + off:(hh + 1) * QC],
                            start=(ki == 0),
                            stop=(ki == nk - 1),
                        )
                    # drop one filler (a proj-et chunk of the previous qt)
                    # into each off-diagonal slot: its PSUM->SBUF copy lands
                    # where scalar/vector have no attention-critical work
                    if fillers and j < 0:
                        fillers.popleft()()
                for hh in range(2):
                    # rows 0..63 are o^T, row 64 is the denominator
                    # (reciprocal_approx_fast misreads PSUM -> copy first)
                    dcp = small.tile(
                        [1, QC], F32, tag="dcp", name=f"dcp{qt}{pair}{hh}"
                    )
                    nc.vector.tensor_copy(dcp[:], pvs[hh][HD:HD + 1, :])
                    rden = small.tile(
                        [1, QC], F32, tag="rden", name=f"rden{qt}{pair}{hh}"
                    )
                    nc.vector.reciprocal_approx_fast(rden[:], dcp[:])
                    bden = small.tile(
                        [64, QC], F32, tag="bden", name=f"bden{qt}{pair}{hh}"
                    )
                    nc.gpsimd.partition_broadcast(bden[:], rden[:])
                    nc.vector.tensor_mul(
                        ot_sb[ht][slice(64 * hh, 64 * hh + 64), q0:q0 + QC],
                        pvs[hh][0:HD, :], bden[:],
                    )

            ystg_cm = tc.tile_pool(name="ystg", bufs=2)
            ystg = ystg_cm.__enter__()

            from collections import deque

            def proj_fillers(qt, split_copy=False):
                """Per-et closures: 2 proj matmuls + staged fp16 copy, and
                the yt DMA once the last chunk lands."""
                q0 = qt * QC
                ys = ystg.tile([P, 8 * QC], F16, tag="ys", name=f"ys{qt}")
                ys_src = ys[:].rearrange("p (e c) -> p e c", e=8)
                yt_dst = yt[:, q0:q0 + QC].rearrange("(e p) c -> p e c", e=8)

                def mk(et):
                    def emit():
                        ps = ps_pv.tile([P, QC], F32, tag="pv", name=f"yp{qt}{et}")
                        for i in range(2):
                            nc.tensor.matmul(
                                ps[:],
                                wp_sb[i][:, et * P:(et + 1) * P],
                                ot_sb[i][:, q0:q0 + QC],
                                start=(i == 0),
                                stop=(i == 1),
                            )
                        dst = ys[:, et * QC:(et + 1) * QC]
                        if split_copy and et % 2 == 0:
                            nc.scalar.copy(dst, ps[:])
                        else:
                            nc.vector.tensor_copy(dst, ps[:])
                        if et == 7:
                            if split_copy:  # final qt: 4 chunks, 2 queues
                                for c in range(4):
                                    eng = nc.sync if c % 2 == 0 else nc.gpsimd
                                    eng.dma_start(
                                        yt_dst[:, 2 * c:2 * c + 2, :],
                                        ys_src[:, 2 * c:2 * c + 2, :],
                                    )
                            else:
                                eng = nc.sync if qt % 2 == 0 else nc.gpsimd
                                eng.dma_start(yt_dst, ys_src)
                    return emit

                return deque(mk(et) for et in range(8))

            def emit_proj(qt, split_dma=False):
                for f in proj_fillers(qt, split_copy=split_dma):
                    f()

            # software-pipelined emission: proj(qt)'s per-et chunks drop
            # into the off-diagonal slots of attention(qt+1), where the
            # scalar/vector queues have no attention-critical work; V tiles
            # trickle in between pairs.
            emit_pair(0, 0)
            for st in range(4, 8):
                emit_v(st)
            emit_pair(0, 1)
            for st in range(8, 12):
                emit_v(st)
            f0 = proj_fillers(0)
            emit_pair(1, 0, fillers=f0)
            for f in f0:  # any leftovers (nk-4 slots may be < 8)
                f()
            for st in range(12, 16):
                emit_v(st)
            f0b = deque()
            emit_pair(1, 1, fillers=f0b)
            f1 = proj_fillers(1)
            emit_pair(2, 0, fillers=f1)
            for f in f1:
                f()
            emit_pair(2, 1)
            f2 = proj_fillers(2)
            emit_pair(3, 0, fillers=f2)
            for f in f2:
                f()
            emit_pair(3, 1, split_exp=True)
            emit_proj(3, split_dma=True)

            ystg_cm.__exit__(None, None, None)
            small_cm.__exit__(None, None, None)
            work_cm.__exit__(None, None, None)
            xtp_cm.__exit__(None, None, None)

    nc.compile()
    return nc


def _shard_inputs(x, w_qkv, b_qkv, w_proj):
    """Build the per-core input maps."""
    in_maps = []
    kk = np.arange(P)[:, None]
    qq = np.arange(P)[None, :]
    import ml_dtypes
    mdt = ml_dtypes.bfloat16
    # one strict-lower-triangle pattern serves every diagonal tile: within
    # the window starting at col 128j, col c is masked iff c < key row p
    tri = (qq >= kk).astype(mdt)
    masks_np = np.concatenate([tri, tri], axis=1)  # duplicated per head
    for c in range(NCORES):
        b, g = divmod(c, 4)
        e0 = g * HPC * HD  # 256*g
        xt_np = np.ascontiguousarray(x[b].T)
        q_rows = w_qkv[e0:e0 + HPC * HD]            # [256, 1024]
        k_rows = w_qkv[D + e0:D + e0 + HPC * HD]
        wqk_np = np.concatenate([q_rows.T, k_rows.T], 1)  # [1024, 512]
        wv_np = np.ascontiguousarray(
            w_qkv[2 * D + e0:2 * D + e0 + HPC * HD].T)    # [1024, 256]
        bqk_np = np.stack(
            [b_qkv[e0:e0 + P], b_qkv[e0 + P:e0 + 2 * P],
             b_qkv[D + e0:D + e0 + P], b_qkv[D + e0 + P:D + e0 + 2 * P]], 1
        ).astype(np.float32)
        wp_np = np.ascontiguousarray(w_proj[:, e0:e0 + HPC * HD].T)  # [256, 1024]
        in_maps.append({
            "xt": np.ascontiguousarray(xt_np.astype(mdt)),
            "wqk": np.ascontiguousarray(wqk_np.astype(mdt)),
            "wv": wv_np.astype(mdt),
            "bqk": np.ascontiguousarray(bqk_np),
            "masks": masks_np,
            "wp": wp_np.astype(mdt),
        })
    return in_maps


def _run(inputs, trace=False, trace_kwargs=None):
    global _PROGRAM
    if _PROGRAM is None:
        _PROGRAM = _build_program()
    nc = _PROGRAM
    x = np.asarray(inputs["x"], np.float32)
    w_qkv = np.asarray(inputs["w_qkv"], np.float32)
    b_qkv = np.asarray(inputs["b_qkv"], np.float32)
    w_proj = np.asarray(inputs["w_proj"], np.float32)
    b_proj = np.asarray(inputs["b_proj"], np.float32)
    in_maps = _shard_inputs(x, w_qkv, b_qkv, w_proj)
    res = run_bass_kernel_spmd(
        nc, in_maps, core_ids=list(range(NCORES)),
        trace=trace, **(trace_kwargs or {}),
    )
    y = np.zeros((B, S, D), np.float32)
    for c in range(NCORES):
        y[c // 4] += res.results[c]["yt"].astype(np.float32).T
    # v-bias commutes through softmax: fold w_proj @ b_v into the output bias
    y += b_proj + w_proj @ b_qkv[2 * D:]
    return y, res


def kernel(**inputs):
    y, _ = _run(inputs)
    return y


# revision 16
# speedup vs baseline: 1.0035x; 1.0035x over previous
"""Causal self-attention (B=2, S=2048, D=1024, H=16) on 8 Trainium2 cores.

Sharding: batch x head-group. Core c handles batch c//4 and heads
[4*(c%4), 4*(c%4)+4). Each core computes q/k/v projections for its head
slice, causal flash-attention (transposed layout, no max-subtraction --
scores are bounded ~9), and a row-parallel partial output projection.
The host transposes/sums the 8 partial outputs and adds the folded bias
(b_proj + w_proj @ b_v -- the v bias commutes through softmax).

All attention + projection matmuls run in bf16 (1 cyc/row on the PE).
Partial outputs are written as fp16 (halves the write traffic; partials
are |y|<~10 so fp16 rounding is ~4e-4 absolute per partial).
"""

import sys

import numpy as np

try:
    import concourse.bass as bass  # noqa: F401
except ImportError:  # fallback for environments without the site hook
    sys.path.insert(0, "/opt/trn_rl_repo")

import concourse.bacc as bacc
import concourse.bass as bass
import concourse.mybir as mybir
from concourse import tile
from concourse.bass_utils import run_bass_kernel_spmd

B, S, D, H = 2, 2048, 1024, 16
HD = D // H  # 64
SCALE = 1.0 / np.sqrt(HD)  # 0.125
HPC = 4          # heads per core
NCORES = 8
P = 128          # partitions
QC = 512         # query chunk (matmul free dim)
NQ = S // QC     # 4 query chunks
NK = S // P      # 16 key tiles
ND = D // P      # 8 d tiles
F32 = mybir.dt.float32
F16 = mybir.dt.float16
BF16 = mybir.dt.bfloat16
ATT_DT = BF16
VPAD = 336                      # v tile cols: 4*65 rounded up so every
                                # head slice can read a full 128-col lhsT
N_WARM = 4                      # dummy matmuls to ramp the PE p-state

_PROGRAM = None


def _build_program():
    """Build the SPMD Bass program (same NEFF for all 8 cores)."""
    nc = bacc.Bacc(None, target_bir_lowering=False)

    xt = nc.declare_dram_parameter("xt", [D, S], ATT_DT, isOutput=False)
    wqk = nc.declare_dram_parameter("wqk", [P, 4 * ND * P], ATT_DT, isOutput=False)
    wv = nc.declare_dram_parameter("wv", [D, HPC * HD], ATT_DT, isOutput=False)
    bqk = nc.declare_dram_parameter("bqk", [P, 4], F32, isOutput=False)
    masks = nc.declare_dram_parameter("masks", [P, 2 * P], ATT_DT, isOutput=False)
    wp = nc.declare_dram_parameter("wp", [HPC * HD, D], ATT_DT, isOutput=False)
    yt = nc.declare_dram_parameter("yt", [D, S], F16, isOutput=True)

    VW = HPC * HD  # 256 cols of v (no bias/ones columns in DRAM)

    with tile.TileContext(nc) as tc:
        with (
            tc.tile_pool(name="const", bufs=1) as const,
            tc.tile_pool(name="big", bufs=1) as bigp,
            tc.tile_pool(name="ps_mm", bufs=2, space="PSUM") as ps_mm,
            tc.tile_pool(name="ps_pv", bufs=4, space="PSUM") as ps_pv,
        ):
            xtp_cm = tc.tile_pool(name="xtp", bufs=1)
            xtp = xtp_cm.__enter__()

            # explicit 2-queue DMA schedule (sync + gpsimd; scalar must
            # stay free for activations). Each dma_start is striped over
            # all 16 physical DMA engines, so fewer+bigger transfers win.
            # ---- PE warm-up: dummy matmuls on a memset tile ramp the
            # p-state while the first DMAs are in flight ----
            warm_sb = const.tile([P, QC], ATT_DT, tag="warm")
            nc.vector.memset(warm_sb[:], 0.0)
            warm_ps = ps_mm.tile([P, QC], F32, tag="mm", name="warm")
            for _ in range(N_WARM):
                nc.tensor.matmul(warm_ps[:], warm_sb[:, 0:P], warm_sb[:],
                                 start=True, stop=True)

            wqk_all = const.tile([P, 4 * ND * P], ATT_DT, tag="wqk")
            xt_sb = [
                xtp.tile([P, S], ATT_DT, tag=f"xt{dt}", name=f"xts{dt}")
                for dt in range(ND)
            ]
            bqk_sb = const.tile([P, 4], F32, tag="bqk")
            nc.gpsimd.dma_start(bqk_sb[:], bqk[:])
            # wqk stored partition-major + et-major in DRAM, so each
            # et-quarter is one fully-contiguous transfer; the first q/k
            # chains start after a quarter of wqk has landed
            NDP = ND * P
            for et in range(4):
                eng = nc.sync if et % 2 == 0 else nc.gpsimd
                eng.dma_start(wqk_all[:, et * NDP:(et + 1) * NDP],
                              wqk[:, et * NDP:(et + 1) * NDP])
            for dt in range(ND):
                eng = nc.sync if dt % 2 == 0 else nc.gpsimd
                eng.dma_start(xt_sb[dt][:, 0:QC], xt[dt * P:(dt + 1) * P, 0:QC])
            # rest of x: sc1 first (consumed next), then sc2+sc3
            for dt in range(ND):
                eng = nc.sync if dt % 2 == 0 else nc.gpsimd
                eng.dma_start(xt_sb[dt][:, QC:2 * QC],
                              xt[dt * P:(dt + 1) * P, QC:2 * QC])
            for dt in range(ND):
                eng = nc.sync if dt % 2 == 0 else nc.gpsimd
                eng.dma_start(xt_sb[dt][:, 2 * QC:S],
                              xt[dt * P:(dt + 1) * P, 2 * QC:S])
            wv_all = const.tile([P, ND * VW], ATT_DT, tag="wv")
            wv_sb = [wv_all[:, dt * VW:(dt + 1) * VW] for dt in range(ND)]
            nc.gpsimd.dma_start(
                wv_all[:].rearrange("p (d c) -> p d c", d=ND),
                wv[:].rearrange("(d p) c -> p d c", d=ND),
            )
            masks_sb = const.tile([P, 2 * P], ATT_DT, tag="masks")
            nc.sync.dma_start(masks_sb[:], masks[:])
            wp_all = const.tile([P, 2 * D], ATT_DT, tag="wp")
            wp_sb = [wp_all[:, i * D:(i + 1) * D] for i in range(2)]
            nc.gpsimd.dma_start(
                wp_all[:].rearrange("p (i c) -> p i c", i=2),
                wp[:].rearrange("(i p) c -> p i c", i=2),
            )

            # ---- persistent intermediates ----
            qt_sb = [bigp.tile([P, S], ATT_DT, tag=f"qt{i}", name=f"qt{i}") for i in range(2)]
            kt_sb = [bigp.tile([P, S], ATT_DT, tag=f"kt{i}", name=f"kt{i}") for i in range(2)]
            v_sb = [bigp.tile([P, VPAD], ATT_DT, tag=f"v{i}", name=f"v{i}") for i in range(NK)]
            ot_sb = [bigp.tile([P, S], ATT_DT, tag=f"ot{i}", name=f"ot{i}") for i in range(2)]

            # ones everywhere except the 4x64 blocks the copies fill:
            # column 65h+64 of each head block stays 1 => the pv matmul's
            # 65-stride window trick yields the softmax denominator row.
            for st in range(NK):
                nc.gpsimd.memset(v_sb[st][:], 1.0)

            # ================= phase 1: q/k projections =================
            for sc in range(NQ):
                for et in range(4):  # 0,1: q heads (0,1),(2,3); 2,3: k heads
                    ps = ps_mm.tile([P, QC], F32, tag="mm", name=f"qk{sc}{et}")
                    for dt in range(ND):
                        nc.tensor.matmul(
                            ps[:],
                            wqk_all[:, et * (ND * P) + dt * P:
                                    et * (ND * P) + (dt + 1) * P],
                            xt_sb[dt][:, sc * QC:(sc + 1) * QC],
                            start=(dt == 0),
                            stop=(dt == ND - 1),
                        )
                    dest = (qt_sb if et < 2 else kt_sb)[et % 2]
                    dst_ap = dest[:, sc * QC:(sc + 1) * QC]
                    nc.vector.tensor_scalar_add(dst_ap, ps[:], bqk_sb[:, et:et + 1])

            # ================= phase 1b: v projection =================
            def emit_v(st):
                ps = ps_mm.tile([P, VW], F32, tag="mm", name=f"vp{st}")
                for dt in range(ND):
                    nc.tensor.matmul(
                        ps[:],
                        xt_sb[dt][:, st * P:(st + 1) * P],
                        wv_sb[dt][:],
                        start=(dt == 0),
                        stop=(dt == ND - 1),
                    )
                # scatter the 4 64-wide head blocks into the 65-stride
                # layout, skipping the ones columns
                dst = v_sb[st][:, 0:4 * 65].rearrange("p (h d) -> p h d", h=4)[:, :, 0:HD]
                src = ps[:].rearrange("p (h d) -> p h d", h=4)
                if st % 2 == 0:
                    nc.scalar.copy(dst, src)
                else:
                    nc.vector.tensor_copy(dst, src)

            for st in range(4):
                emit_v(st)

            work_cm = tc.tile_pool(name="work", bufs=6)
            work = work_cm.__enter__()
            small_cm = tc.tile_pool(name="small", bufs=3)
            small = small_cm.__enter__()

            # ================= phase 2: attention =================
            def emit_pair(qt, pair, fillers=None, split_exp=False):
                q0 = qt * QC
                nk = (qt + 1) * (QC // P)  # causal: k tiles 0..nk-1
                ht = pair
                pvs = [
                    ps_pv.tile([P, QC], F32, tag="pv", name=f"pv{qt}{pair}{hh}")
                    for hh in range(2)
                ]
                for ki, kb in enumerate(range(nk)):
                    j = kb - qt * (QC // P)
                    # diagonal strip: columns < 128*j are fully masked
                    off = 0 if j < 0 else P * j
                    w = QC - off
                    st2 = ps_mm.tile(
                        [P, 2 * QC], F32, tag="mm", name=f"st{qt}{pair}{kb}"
                    )
                    for hh in range(2):
                        nc.tensor.matmul(
                            st2[:, hh * QC + off:(hh + 1) * QC],
                            kt_sb[ht][slice(64 * hh, 64 * hh + 64),
                                      kb * P:(kb + 1) * P],
                            qt_sb[ht][slice(64 * hh, 64 * hh + 64),
                                      q0 + off:q0 + QC],
                            start=True, stop=True,
                            tile_position=(64 * hh, 0),
                        )
                    ex = work.tile(
                        [P, 2 * QC], ATT_DT, tag="ex", name=f"ex{qt}{pair}{kb}"
                    )
                    st3 = st2[:].rearrange("p (h q) -> p h q", h=2)[:, :, off:]
                    ex3 = ex[:].rearrange("p (h q) -> p h q", h=2)[:, :, off:]
                    if split_exp and j >= 0:
                        # final pair: halve the serial exp latency per tile
                        for hh in range(2):
                            nc.scalar.activation(
                                ex3[:, hh:hh + 1, :], st3[:, hh:hh + 1, :],
                                mybir.ActivationFunctionType.Exp,
                                scale=float(SCALE),
                            )
                    else:
                        nc.scalar.activation(
                            ex3, st3,
                            mybir.ActivationFunctionType.Exp,
                            scale=float(SCALE),
                        )
                    if j >= 0:
                        # only the leading 128 columns of the window straddle
                        # the diagonal; the rest is fully unmasked
                        exm = ex[:].rearrange("p (h q) -> p h q", h=2)[:, :, off:off + P]
                        m3 = masks_sb[:].rearrange("p (h q) -> p h q", h=2)
                        nc.vector.tensor_mul(exm, exm, m3)
                    for hh in range(2):
                        h = 2 * pair + hh
                        nc.tensor.matmul(
                            pvs[hh][:, off:],
                            v_sb[kb][:, h * (HD + 1):h * (HD + 1) + P],
                            ex[:, hh * QC + off:(hh + 1) * QC],
                            start=(ki == 0),
                            stop=(ki == nk - 1),
                        )
                    # drop one filler (a proj-et chunk of the previous qt)
                    # into each off-diagonal slot: its PSUM->SBUF copy lands
                    # where scalar/vector have no attention-critical work
                    if fillers and j < 0:
                        fillers.popleft()()
                for hh in range(2):
                    # rows 0..63 are o^T, row 64 is the denominator
                    # (reciprocal_approx_fast misreads PSUM -> copy first)
                    dcp = small.tile(
                        [1, QC], F32, tag="dcp", name=f"dcp{qt}{pair}{hh}"
                    )
                    nc.vector.tensor_copy(dcp[:], pvs[hh][HD:HD + 1, :])
                    rden = small.tile(
                        [1, QC], F32, tag="rden", name=f"rden{qt}{pair}{hh}"
                    )
                    nc.vector.reciprocal_approx_fast(rden[:], dcp[:])
                    bden = small.tile(
                        [64, QC], F32, tag="bden", name=f"bden{qt}{pair}{hh}"
                    )
                    nc.gpsimd.partition_broadcast(bden[:], rden[:])
                    nc.vector.tensor_mul(
                        ot_sb[ht][slice(64 * hh, 64 * hh + 64), q0:q0 + QC],
                        pvs[hh][0:HD, :], bden[:],
                    )

            ystg_cm = tc.tile_pool(name="ystg", bufs=2)
            ystg = ystg_cm.__enter__()

            from collections import deque

            def proj_fillers(qt, split_copy=False):
                """Per-et closures: 2 proj matmuls + staged fp16 copy, and
                the yt DMA once the last chunk lands."""
                q0 = qt * QC
                ys = ystg.tile([P, 8 * QC], F16, tag="ys", name=f"ys{qt}")
                ys_src = ys[:].rearrange("p (e c) -> p e c", e=8)
                yt_dst = yt[:, q0:q0 + QC].rearrange("(e p) c -> p e c", e=8)

                def mk(et):
                    def emit():
                        ps = ps_pv.tile([P, QC], F32, tag="pv", name=f"yp{qt}{et}")
                        for i in range(2):
                            nc.tensor.matmul(
                                ps[:],
                                wp_sb[i][:, et * P:(et + 1) * P],
                                ot_sb[i][:, q0:q0 + QC],
                                start=(i == 0),
                                stop=(i == 1),
                            )
                        dst = ys[:, et * QC:(et + 1) * QC]
                        if split_copy and et % 2 == 0:
                            nc.scalar.copy(dst, ps[:])
                        else:
                            nc.vector.tensor_copy(dst, ps[:])
                        if et == 7:
                            if split_copy:  # final qt: 4 chunks, 2 queues
                                for c in range(4):
                                    eng = nc.sync if c % 2 == 0 else nc.gpsimd
                                    eng.dma_start(
                                        yt_dst[:, 2 * c:2 * c + 2, :],
                                        ys_src[:, 2 * c:2 * c + 2, :],
                                    )
                            else:
                                eng = nc.sync if qt % 2 == 0 else nc.gpsimd
                                eng.dma_start(yt_dst, ys_src)
                    return emit

                return deque(mk(et) for et in range(8))

            def emit_proj(qt, split_dma=False):
                for f in proj_fillers(qt, split_copy=split_dma):
                    f()

            # software-pipelined emission: proj(qt)'s per-et chunks drop
            # into the off-diagonal slots of attention(qt+1), where the
            # scalar/vector queues have no attention-critical work; V tiles
            # trickle in between pairs.
            emit_pair(0, 0)
            for st in range(4, 8):
                emit_v(st)
            emit_pair(0, 1)
            for st in range(8, 12):
                emit_v(st)
            f0 = proj_fillers(0)
            emit_pair(1, 0, fillers=f0)
            for f in f0:  # any leftovers (nk-4 slots may be < 8)
                f()
            for st in range(12, 16):
                emit_v(st)
            f0b = deque()
            emit_pair(1, 1, fillers=f0b)
            f1 = proj_fillers(1)
            emit_pair(2, 0, fillers=f1)
            for f in f1:
                f()
            emit_pair(2, 1)
            f2 = proj_fillers(2)
            emit_pair(3, 0, fillers=f2)
            for f in f2:
                f()
            emit_pair(3, 1, split_exp=True)
            emit_proj(3, split_dma=True)

            ystg_cm.__exit__(None, None, None)
            small_cm.__exit__(None, None, None)
            work_cm.__exit__(None, None, None)
            xtp_cm.__exit__(None, None, None)

    nc.compile()
    return nc


def _shard_inputs(x, w_qkv, b_qkv, w_proj):
    """Build the per-core input maps."""
    in_maps = []
    kk = np.arange(P)[:, None]
    qq = np.arange(P)[None, :]
    import ml_dtypes
    mdt = ml_dtypes.bfloat16
    # one strict-lower-triangle pattern serves every diagonal tile: within
    # the window starting at col 128j, col c is masked iff c < key row p
    tri = (qq >= kk).astype(mdt)
    masks_np = np.concatenate([tri, tri], axis=1)  # duplicated per head
    for c in range(NCORES):
        b, g = divmod(c, 4)
        e0 = g * HPC * HD  # 256*g
        xt_np = np.ascontiguousarray(x[b].T)
        q_rows = w_qkv[e0:e0 + HPC * HD]            # [256, 1024]
        k_rows = w_qkv[D + e0:D + e0 + HPC * HD]
        wqk_dm = np.concatenate([q_rows.T, k_rows.T], 1)  # [1024, 512]
        # -> partition-major, et-major: [128 p, 4 et, 8 dt, 128 c]
        wqk_np = (wqk_dm.reshape(8, 128, 4, 128)
                  .transpose(1, 2, 0, 3).reshape(128, 4096))
        wv_np = np.ascontiguousarray(
            w_qkv[2 * D + e0:2 * D + e0 + HPC * HD].T)    # [1024, 256]
        bqk_np = np.stack(
            [b_qkv[e0:e0 + P], b_qkv[e0 + P:e0 + 2 * P],
             b_qkv[D + e0:D + e0 + P], b_qkv[D + e0 + P:D + e0 + 2 * P]], 1
        ).astype(np.float32)
        wp_np = np.ascontiguousarray(w_proj[:, e0:e0 + HPC * HD].T)  # [256, 1024]
        in_maps.append({
            "xt": np.ascontiguousarray(xt_np.astype(mdt)),
            "wqk": np.ascontiguousarray(wqk_np.astype(mdt)),
            "wv": wv_np.astype(mdt),
            "bqk": np.ascontiguousarray(bqk_np),
            "masks": masks_np,
            "wp": wp_np.astype(mdt),
        })
    return in_maps


def _run(inputs, trace=False, trace_kwargs=None):
    global _PROGRAM
    if _PROGRAM is None:
        _PROGRAM = _build_program()
    nc = _PROGRAM
    x = np.asarray(inputs["x"], np.float32)
    w_qkv = np.asarray(inputs["w_qkv"], np.float32)
    b_qkv = np.asarray(inputs["b_qkv"], np.float32)
    w_proj = np.asarray(inputs["w_proj"], np.float32)
    b_proj = np.asarray(inputs["b_proj"], np.float32)
    in_maps = _shard_inputs(x, w_qkv, b_qkv, w_proj)
    res = run_bass_kernel_spmd(
        nc, in_maps, core_ids=list(range(NCORES)),
        trace=trace, **(trace_kwargs or {}),
    )
    y = np.zeros((B, S, D), np.float32)
    for c in range(NCORES):
        y[c // 4] += res.results[c]["yt"].astype(np.float32).T
    # v-bias commutes through softmax: fold w_proj @ b_v into the output bias
    y += b_proj + w_proj @ b_qkv[2 * D:]
    return y, res


def kernel(**inputs):
    y, _ = _run(inputs)
    return y


# revision 17
# speedup vs baseline: 1.0278x; 1.0243x over previous
"""Causal self-attention (B=2, S=2048, D=1024, H=16) on 8 Trainium2 cores.

Sharding: batch x head-group. Core c handles batch c//4 and heads
[4*(c%4), 4*(c%4)+4). Each core computes q/k/v projections for its head
slice, causal flash-attention (transposed layout, no max-subtraction --
scores are bounded ~9), and a row-parallel partial output projection.
The host transposes/sums the 8 partial outputs and adds the folded bias
(b_proj + w_proj @ b_v -- the v bias commutes through softmax).

All attention + projection matmuls run in bf16 (1 cyc/row on the PE).
Partial outputs are written as fp16 (halves the write traffic; partials
are |y|<~10 so fp16 rounding is ~4e-4 absolute per partial).
"""

import sys

import numpy as np

try:
    import concourse.bass as bass  # noqa: F401
except ImportError:  # fallback for environments without the site hook
    sys.path.insert(0, "/opt/trn_rl_repo")

import concourse.bacc as bacc
import concourse.bass as bass
import concourse.mybir as mybir
from concourse import tile
from concourse.bass_utils import run_bass_kernel_spmd

B, S, D, H = 2, 2048, 1024, 16
HD = D // H  # 64
SCALE = 1.0 / np.sqrt(HD)  # 0.125
HPC = 4          # heads per core
NCORES = 8
P = 128          # partitions
QC = 512         # query chunk (matmul free dim)
NQ = S // QC     # 4 query chunks
NK = S // P      # 16 key tiles
ND = D // P      # 8 d tiles
F32 = mybir.dt.float32
F16 = mybir.dt.float16
BF16 = mybir.dt.bfloat16
ATT_DT = BF16
VPAD = 336                      # v tile cols: 4*65 rounded up so every
                                # head slice can read a full 128-col lhsT
N_WARM = 3                      # dummy matmuls to ramp the PE p-state

_PROGRAM = None


def _build_program():
    """Build the SPMD Bass program (same NEFF for all 8 cores)."""
    nc = bacc.Bacc(None, target_bir_lowering=False)

    xt = nc.declare_dram_parameter("xt", [D, S], ATT_DT, isOutput=False)
    wqk = nc.declare_dram_parameter("wqk", [P, 4 * ND * P], ATT_DT, isOutput=False)
    wv = nc.declare_dram_parameter("wv", [D, HPC * HD], ATT_DT, isOutput=False)
    bqk = nc.declare_dram_parameter("bqk", [P, 4], F32, isOutput=False)
    masks = nc.declare_dram_parameter("masks", [P, 2 * P], ATT_DT, isOutput=False)
    wp = nc.declare_dram_parameter("wp", [HPC * HD, D], ATT_DT, isOutput=False)
    yt = nc.declare_dram_parameter("yt", [D, S], F16, isOutput=True)

    VW = HPC * HD  # 256 cols of v (no bias/ones columns in DRAM)

    with tile.TileContext(nc) as tc:
        with (
            tc.tile_pool(name="const", bufs=1) as const,
            tc.tile_pool(name="big", bufs=1) as bigp,
            tc.tile_pool(name="ps_mm", bufs=2, space="PSUM") as ps_mm,
            tc.tile_pool(name="ps_pv", bufs=4, space="PSUM") as ps_pv,
        ):
            xtp_cm = tc.tile_pool(name="xtp", bufs=1)
            xtp = xtp_cm.__enter__()

            # explicit 2-queue DMA schedule (sync + gpsimd; scalar must
            # stay free for activations). Each dma_start is striped over
            # all 16 physical DMA engines, so fewer+bigger transfers win.
            # ---- PE warm-up: dummy matmuls on a memset tile ramp the
            # p-state while the first DMAs are in flight ----
            warm_sb = const.tile([P, QC], ATT_DT, tag="warm")
            nc.vector.memset(warm_sb[:], 0.0)
            warm_ps = ps_mm.tile([P, QC], F32, tag="mm", name="warm")
            for _ in range(N_WARM):
                nc.tensor.matmul(warm_ps[:], warm_sb[:, 0:P], warm_sb[:],
                                 start=True, stop=True)

            wqk_all = const.tile([P, 4 * ND * P], ATT_DT, tag="wqk")
            xt_all = xtp.tile([P, ND * S], ATT_DT, tag="xt")
            xt_sb = [xt_all[:, dt * S:(dt + 1) * S] for dt in range(ND)]
            xt_src = xt[:].rearrange("(d p) c -> p d c", d=ND)
            xt_dst = xt_all[:].rearrange("p (d c) -> p d c", d=ND)
            bqk_sb = const.tile([P, 4], F32, tag="bqk")
            nc.gpsimd.dma_start(bqk_sb[:], bqk[:])
            # critical prefix: q-head wqk quarters + the first x chunk,
            # two issues per queue, everything else behind them
            NDP = ND * P
            nc.sync.dma_start(wqk_all[:, 0:NDP], wqk[:, 0:NDP])
            nc.gpsimd.dma_start(wqk_all[:, NDP:2 * NDP], wqk[:, NDP:2 * NDP])
            nc.sync.dma_start(xt_dst[:, 0:4, 0:QC], xt_src[:, 0:4, 0:QC])
            nc.gpsimd.dma_start(xt_dst[:, 4:8, 0:QC], xt_src[:, 4:8, 0:QC])
            # k-head wqk quarters, then the rest of x in rising-urgency order
            nc.sync.dma_start(wqk_all[:, 2 * NDP:3 * NDP], wqk[:, 2 * NDP:3 * NDP])
            nc.gpsimd.dma_start(wqk_all[:, 3 * NDP:4 * NDP], wqk[:, 3 * NDP:4 * NDP])
            nc.sync.dma_start(xt_dst[:, 0:4, QC:2 * QC], xt_src[:, 0:4, QC:2 * QC])
            nc.gpsimd.dma_start(xt_dst[:, 4:8, QC:2 * QC], xt_src[:, 4:8, QC:2 * QC])
            nc.sync.dma_start(xt_dst[:, 0:4, 2 * QC:S], xt_src[:, 0:4, 2 * QC:S])
            nc.gpsimd.dma_start(xt_dst[:, 4:8, 2 * QC:S], xt_src[:, 4:8, 2 * QC:S])
            wv_all = const.tile([P, ND * VW], ATT_DT, tag="wv")
            wv_sb = [wv_all[:, dt * VW:(dt + 1) * VW] for dt in range(ND)]
            nc.gpsimd.dma_start(
                wv_all[:].rearrange("p (d c) -> p d c", d=ND),
                wv[:].rearrange("(d p) c -> p d c", d=ND),
            )
            masks_sb = const.tile([P, 2 * P], ATT_DT, tag="masks")
            nc.sync.dma_start(masks_sb[:], masks[:])
            wp_all = const.tile([P, 2 * D], ATT_DT, tag="wp")
            wp_sb = [wp_all[:, i * D:(i + 1) * D] for i in range(2)]
            nc.gpsimd.dma_start(
                wp_all[:].rearrange("p (i c) -> p i c", i=2),
                wp[:].rearrange("(i p) c -> p i c", i=2),
            )

            # ---- persistent intermediates ----
            qt_sb = [bigp.tile([P, S], ATT_DT, tag=f"qt{i}", name=f"qt{i}") for i in range(2)]
            kt_sb = [bigp.tile([P, S], ATT_DT, tag=f"kt{i}", name=f"kt{i}") for i in range(2)]
            v_sb = [bigp.tile([P, VPAD], ATT_DT, tag=f"v{i}", name=f"v{i}") for i in range(NK)]
            ot_sb = [bigp.tile([P, S], ATT_DT, tag=f"ot{i}", name=f"ot{i}") for i in range(2)]

            # ones everywhere except the 4x64 blocks the copies fill:
            # column 65h+64 of each head block stays 1 => the pv matmul's
            # 65-stride window trick yields the softmax denominator row.
            for st in range(NK):
                nc.gpsimd.memset(v_sb[st][:], 1.0)

            # ================= phase 1: q/k projections =================
            for sc in range(NQ):
                for et in range(4):  # 0,1: q heads (0,1),(2,3); 2,3: k heads
                    ps = ps_mm.tile([P, QC], F32, tag="mm", name=f"qk{sc}{et}")
                    for dt in range(ND):
                        nc.tensor.matmul(
                            ps[:],
                            wqk_all[:, et * (ND * P) + dt * P:
                                    et * (ND * P) + (dt + 1) * P],
                            xt_sb[dt][:, sc * QC:(sc + 1) * QC],
                            start=(dt == 0),
                            stop=(dt == ND - 1),
                        )
                    dest = (qt_sb if et < 2 else kt_sb)[et % 2]
                    dst_ap = dest[:, sc * QC:(sc + 1) * QC]
                    nc.vector.tensor_scalar_add(dst_ap, ps[:], bqk_sb[:, et:et + 1])

            # ================= phase 1b: v projection =================
            def emit_v(st):
                ps = ps_mm.tile([P, VW], F32, tag="mm", name=f"vp{st}")
                for dt in range(ND):
                    nc.tensor.matmul(
                        ps[:],
                        xt_sb[dt][:, st * P:(st + 1) * P],
                        wv_sb[dt][:],
                        start=(dt == 0),
                        stop=(dt == ND - 1),
                    )
                # scatter the 4 64-wide head blocks into the 65-stride
                # layout, skipping the ones columns
                dst = v_sb[st][:, 0:4 * 65].rearrange("p (h d) -> p h d", h=4)[:, :, 0:HD]
                src = ps[:].rearrange("p (h d) -> p h d", h=4)
                if st % 2 == 0:
                    nc.scalar.copy(dst, src)
                else:
                    nc.vector.tensor_copy(dst, src)

            for st in range(4):
                emit_v(st)

            work_cm = tc.tile_pool(name="work", bufs=6)
            work = work_cm.__enter__()
            small_cm = tc.tile_pool(name="small", bufs=3)
            small = small_cm.__enter__()

            # ================= phase 2: attention =================
            def emit_pair(qt, pair, fillers=None, split_exp=False):
                q0 = qt * QC
                nk = (qt + 1) * (QC // P)  # causal: k tiles 0..nk-1
                ht = pair
                pvs = [
                    ps_pv.tile([P, QC], F32, tag="pv", name=f"pv{qt}{pair}{hh}")
                    for hh in range(2)
                ]
                for ki, kb in enumerate(range(nk)):
                    j = kb - qt * (QC // P)
                    # diagonal strip: columns < 128*j are fully masked
                    off = 0 if j < 0 else P * j
                    w = QC - off
                    st2 = ps_mm.tile(
                        [P, 2 * QC], F32, tag="mm", name=f"st{qt}{pair}{kb}"
                    )
                    for hh in range(2):
                        nc.tensor.matmul(
                            st2[:, hh * QC + off:(hh + 1) * QC],
                            kt_sb[ht][slice(64 * hh, 64 * hh + 64),
                                      kb * P:(kb + 1) * P],
                            qt_sb[ht][slice(64 * hh, 64 * hh + 64),
                                      q0 + off:q0 + QC],
                            start=True, stop=True,
                            tile_position=(64 * hh, 0),
                        )
                    ex = work.tile(
                        [P, 2 * QC], ATT_DT, tag="ex", name=f"ex{qt}{pair}{kb}"
                    )
                    st3 = st2[:].rearrange("p (h q) -> p h q", h=2)[:, :, off:]
                    ex3 = ex[:].rearrange("p (h q) -> p h q", h=2)[:, :, off:]
                    if split_exp and j >= 0:
                        # final pair: halve the serial exp latency per tile
                        for hh in range(2):
                            nc.scalar.activation(
                                ex3[:, hh:hh + 1, :], st3[:, hh:hh + 1, :],
                                mybir.ActivationFunctionType.Exp,
                                scale=float(SCALE),
                            )
                    else:
                        nc.scalar.activation(
                            ex3, st3,
                            mybir.ActivationFunctionType.Exp,
                            scale=float(SCALE),
                        )
                    if j >= 0:
                        # only the leading 128 columns of the window straddle
                        # the diagonal; the rest is fully unmasked
                        exm = ex[:].rearrange("p (h q) -> p h q", h=2)[:, :, off:off + P]
                        m3 = masks_sb[:].rearrange("p (h q) -> p h q", h=2)
                        nc.vector.tensor_mul(exm, exm, m3)
                    for hh in range(2):
                        h = 2 * pair + hh
                        nc.tensor.matmul(
                            pvs[hh][:, off:],
                            v_sb[kb][:, h * (HD + 1):h * (HD + 1) + P],
                            ex[:, hh * QC + off:(hh + 1) * QC],
                            start=(ki == 0),
                            stop=(ki == nk - 1),
                        )
                    # drop one filler (a proj-et chunk of the previous qt)
                    # into each off-diagonal slot: its PSUM->SBUF copy lands
                    # where scalar/vector have no attention-critical work
                    if fillers and j < 0:
                        fillers.popleft()()
                for hh in range(2):
                    # rows 0..63 are o^T, row 64 is the denominator
                    # (reciprocal_approx_fast misreads PSUM -> copy first)
                    dcp = small.tile(
                        [1, QC], F32, tag="dcp", name=f"dcp{qt}{pair}{hh}"
                    )
                    nc.vector.tensor_copy(dcp[:], pvs[hh][HD:HD + 1, :])
                    rden = small.tile(
                        [1, QC], F32, tag="rden", name=f"rden{qt}{pair}{hh}"
                    )
                    nc.vector.reciprocal_approx_fast(rden[:], dcp[:])
                    bden = small.tile(
                        [64, QC], F32, tag="bden", name=f"bden{qt}{pair}{hh}"
                    )
                    nc.gpsimd.partition_broadcast(bden[:], rden[:])
                    nc.vector.tensor_mul(
                        ot_sb[ht][slice(64 * hh, 64 * hh + 64), q0:q0 + QC],
                        pvs[hh][0:HD, :], bden[:],
                    )

            ystg_cm = tc.tile_pool(name="ystg", bufs=2)
            ystg = ystg_cm.__enter__()

            from collections import deque

            def proj_fillers(qt, split_copy=False):
                """Per-et closures: 2 proj matmuls + staged fp16 copy, and
                the yt DMA once the last chunk lands."""
                q0 = qt * QC
                ys = ystg.tile([P, 8 * QC], F16, tag="ys", name=f"ys{qt}")
                ys_src = ys[:].rearrange("p (e c) -> p e c", e=8)
                yt_dst = yt[:, q0:q0 + QC].rearrange("(e p) c -> p e c", e=8)

                def mk(et):
                    def emit():
                        ps = ps_pv.tile([P, QC], F32, tag="pv", name=f"yp{qt}{et}")
                        for i in range(2):
                            nc.tensor.matmul(
                                ps[:],
                                wp_sb[i][:, et * P:(et + 1) * P],
                                ot_sb[i][:, q0:q0 + QC],
                                start=(i == 0),
                                stop=(i == 1),
                            )
                        dst = ys[:, et * QC:(et + 1) * QC]
                        if et % 2 == 0:
                            nc.scalar.copy(dst, ps[:])
                        else:
                            nc.vector.tensor_copy(dst, ps[:])
                        if et == 7:
                            if split_copy:  # final qt: 4 chunks, 2 queues
                                for c in range(4):
                                    eng = nc.sync if c % 2 == 0 else nc.gpsimd
                                    eng.dma_start(
                                        yt_dst[:, 2 * c:2 * c + 2, :],
                                        ys_src[:, 2 * c:2 * c + 2, :],
                                    )
                            else:
                                eng = nc.sync if qt % 2 == 0 else nc.gpsimd
                                eng.dma_start(yt_dst, ys_src)
                    return emit

                return deque(mk(et) for et in range(8))

            def emit_proj(qt, split_dma=False):
                for f in proj_fillers(qt, split_copy=split_dma):
                    f()

            # software-pipelined emission: proj(qt) goes into the middle
            # of attention(qt+1); V tiles trickle in between pairs.
            emit_pair(0, 0)
            for st in range(4, 8):
                emit_v(st)
            emit_pair(0, 1)
            for st in range(8, 12):
                emit_v(st)
            emit_pair(1, 0)
            emit_proj(0)
            for st in range(12, 16):
                emit_v(st)
            emit_pair(1, 1)
            emit_pair(2, 0)
            emit_proj(1)
            emit_pair(2, 1)
            emit_pair(3, 0)
            emit_proj(2)
            emit_pair(3, 1, split_exp=True)
            emit_proj(3, split_dma=True)

            ystg_cm.__exit__(None, None, None)
            small_cm.__exit__(None, None, None)
            work_cm.__exit__(None, None, None)
            xtp_cm.__exit__(None, None, None)

    nc.compile()
    return nc


def _shard_inputs(x, w_qkv, b_qkv, w_proj):
    """Build the per-core input maps."""
    in_maps = []
    kk = np.arange(P)[:, None]
    qq = np.arange(P)[None, :]
    import ml_dtypes
    mdt = ml_dtypes.bfloat16
    # one strict-lower-triangle pattern serves every diagonal tile: within
    # the window starting at col 128j, col c is masked iff c < key row p
    tri = (qq >= kk).astype(mdt)
    masks_np = np.concatenate([tri, tri], axis=1)  # duplicated per head
    for c in range(NCORES):
        b, g = divmod(c, 4)
        e0 = g * HPC * HD  # 256*g
        xt_np = np.ascontiguousarray(x[b].T)
        q_rows = w_qkv[e0:e0 + HPC * HD]            # [256, 1024]
        k_rows = w_qkv[D + e0:D + e0 + HPC * HD]
        wqk_dm = np.concatenate([q_rows.T, k_rows.T], 1)  # [1024, 512]
        # -> partition-major, et-major: [128 p, 4 et, 8 dt, 128 c]
        wqk_np = (wqk_dm.reshape(8, 128, 4, 128)
                  .transpose(1, 2, 0, 3).reshape(128, 4096))
        wv_np = np.ascontiguousarray(
            w_qkv[2 * D + e0:2 * D + e0 + HPC * HD].T)    # [1024, 256]
        bqk_np = np.stack(
            [b_qkv[e0:e0 + P], b_qkv[e0 + P:e0 + 2 * P],
             b_qkv[D + e0:D + e0 + P], b_qkv[D + e0 + P:D + e0 + 2 * P]], 1
        ).astype(np.float32)
        wp_np = np.ascontiguousarray(w_proj[:, e0:e0 + HPC * HD].T)  # [256, 1024]
        in_maps.append({
            "xt": np.ascontiguousarray(xt_np.astype(mdt)),
            "wqk": np.ascontiguousarray(wqk_np.astype(mdt)),
            "wv": wv_np.astype(mdt),
            "bqk": np.ascontiguousarray(bqk_np),
            "masks": masks_np,
            "wp": wp_np.astype(mdt),
        })
    return in_maps


def _run(inputs, trace=False, trace_kwargs=None):
    global _PROGRAM
    if _PROGRAM is None:
        _PROGRAM = _build_program()
    nc = _PROGRAM
    x = np.asarray(inputs["x"], np.float32)
    w_qkv = np.asarray(inputs["w_qkv"], np.float32)
    b_qkv = np.asarray(inputs["b_qkv"], np.float32)
    w_proj = np.asarray(inputs["w_proj"], np.float32)
    b_proj = np.asarray(inputs["b_proj"], np.float32)
    in_maps = _shard_inputs(x, w_qkv, b_qkv, w_proj)
    res = run_bass_kernel_spmd(
        nc, in_maps, core_ids=list(range(NCORES)),
        trace=trace, **(trace_kwargs or {}),
    )
    y = np.zeros((B, S, D), np.float32)
    for c in range(NCORES):
        y[c // 4] += res.results[c]["yt"].astype(np.float32).T
    # v-bias commutes through softmax: fold w_proj @ b_v into the output bias
    y += b_proj + w_proj @ b_qkv[2 * D:]
    return y, res


def kernel(**inputs):
    y, _ = _run(inputs)
    return y
